# revision 1
# baseline (speedup 1.0000x reference)
"""BPS condition tokenizer (nearest-neighbor argmin + delta encode) on 8 trn2 cores.

Strategy
--------
For each (batch b, basis point p) we need argmin_n ||pc[b,n] - basis[p]||^2.
Equivalently argmax_n s'[p,n] with s' = 2*<basis[p], pc[n]> - |pc[n]|^2, which
is a K=4 augmented matmul:  s' = A^T X  with A = [2*bx; 2*by; 2*bz; -1] and
X = [x; y; z; |p|^2].  Per core (2 of 16 batches, data-parallel over B):

  PE      : 32 p-tiles x 2 batches x 8 chunks of [128 x 512] float32r matmuls
  ScalarE : PSUM -> SBUF copy (fp32)
  VectorE : InstMax (top-8 values / row) + InstMaxIndex (their indices)
  DMA out : top-8 values + indices per row  (the [B,P,N] tensor never leaves PSUM)

float32r runs the PE at 1 col/cycle (4x faster than fp32) at ~tf32 precision;
measured |error| <= 1.5e-3 on s'.  The host then rescores the <=8 candidates
per row in fp64 (exact), falls back to a full-row fp64 scan for rows whose
device top-8 spread is within the noise band (coverage risk), and resolves
knife-edge rows (fp64 top-2 gap < 1e-5, where fp32 reference rounding decides)
with the reference's own jnp ops on just those rows - row-slicing of the
einsum is bitwise-stable, so those rows reproduce the reference argmin bit
for bit.  Final gather/delta/dist assembly also uses the reference's jnp ops.
"""

import numpy as np

import concourse.bass as bass
import concourse.mybir as mybir
from concourse import bacc
from concourse.tile import TileContext
from concourse.bass_utils import run_bass_kernel_spmd

FP32 = mybir.dt.float32

# problem shape (hardcoded per contract)
B, N, D = 16, 4096, 3
P = 4096
NCORES = 8
BPC = B // NCORES          # batches per core
CH = 512                   # matmul moving free dim (1 PSUM bank of fp32)
NPT = P // 128             # basis tiles of 128 rows
NCH = N // CH

# f32r noise band on s' (measured 1.45e-3 max; margin ~1.7x)
COVERAGE_EPS = 2 * 1.25e-3 * 2.0    # spread threshold: 2 * noise, extra margin
KNIFE_EPS = 1e-5                    # fp64 top-2 gap below which fp32 rounding decides

_nc_cache = {}


def _build_program():
    if "nc" in _nc_cache:
        return _nc_cache["nc"]
    nc = bacc.Bacc("TRN2", target_bir_lowering=False, debug=False,
                   num_devices=NCORES)
    A = nc.dram_tensor("A", [4, P], FP32, kind="ExternalInput").ap()
    X = nc.dram_tensor("X", [BPC, 4, N], FP32, kind="ExternalInput").ap()
    vals = nc.dram_tensor("vals", [BPC, NPT, 128, 8], FP32,
                          kind="ExternalOutput").ap()
    idx = nc.dram_tensor("idx", [BPC, NPT, 128, 8], mybir.dt.uint32,
                         kind="ExternalOutput").ap()
    mm_dt = mybir.dt.float32r

    with TileContext(nc) as tc:
        with tc.tile_pool(name="const", bufs=1) as cpool, \
             tc.tile_pool(name="stile", bufs=3) as spool, \
             tc.tile_pool(name="psum", bufs=8, space="PSUM") as ppool, \
             tc.tile_pool(name="out8", bufs=8) as opool:

            A_sb = cpool.tile([4, P], mm_dt, tag="A")
            nc.gpsimd.dma_start(out=A_sb[:, :], in_=A[:, :])  # casts f32 -> f32r
            X_sb = []
            for b in range(BPC):
                xb = cpool.tile([4, N], mm_dt, tag=f"X{b}")
                nc.gpsimd.dma_start(out=xb[:, :], in_=X[b, :, :])
                X_sb.append(xb)

            for pt in range(NPT):
                lhsT = A_sb[:, pt * 128:(pt + 1) * 128]
                for b in range(BPC):
                    stile = spool.tile([128, N], FP32, tag="s")
                    for c in range(NCH):
                        ps = ppool.tile([128, CH], FP32, tag="ps")
                        nc.tensor.matmul(ps[:, :], lhsT,
                                         X_sb[b][:, c * CH:(c + 1) * CH],
                                         start=True, stop=True)
                        nc.scalar.copy(stile[:, c * CH:(c + 1) * CH], ps[:, :])
                    top = opool.tile([128, 8], FP32, tag="top")
                    ids = opool.tile([128, 8], mybir.dt.uint32, tag="ids")
                    nc.vector.max(out=top[:, :], in_=stile[:, :])
                    nc.vector.max_index(out=ids[:, :], in_max=top[:, :],
                                        in_values=stile[:, :])
                    nc.sync.dma_start(out=vals[b, pt, :, :], in_=top[:, :])
                    nc.sync.dma_start(out=idx[b, pt, :, :], in_=ids[:, :])
    nc.compile()
    _nc_cache["nc"] = nc
    return nc


def _run_device(point_cloud, basis, trace=False):
    """Shard over batch, run the bass kernel on 8 cores, return candidate
    indices/values [B, P, 8] plus the BassKernelResults (for profiling)."""
    nc = _build_program()
    A = np.concatenate([2.0 * basis.T, -np.ones((1, P), np.float32)],
                       0).astype(np.float32)
    pc_sq = (point_cloud.astype(np.float32) ** 2).sum(-1)
    X_full = np.concatenate([point_cloud.transpose(0, 2, 1),
                             pc_sq[:, None, :]], 1).astype(np.float32)
    in_maps = [{"A": A, "X": X_full[i * BPC:(i + 1) * BPC]}
               for i in range(NCORES)]
    res = run_bass_kernel_spmd(nc, in_maps, list(range(NCORES)), trace=trace)
    vals = np.stack([res.results[i]["vals"] for i in range(NCORES)])
    idx = np.stack([res.results[i]["idx"] for i in range(NCORES)])
    vals = vals.reshape(B, P, 8)
    idx = idx.reshape(B, P, 8).astype(np.int64)
    return vals, idx, res


def _resolve_indices(point_cloud, basis, vals, idx):
    """Turn device top-8 candidates into the reference's exact argmin."""
    import jax.numpy as jnp

    pc64 = point_cloud.astype(np.float64)
    b64 = basis.astype(np.float64)
    idx = np.clip(idx, 0, N - 1)

    # 1) fp64 rescore of the <=8 candidates per row (vectorized)
    #    d2 = |pc[idx] - basis|^2
    cand = np.take_along_axis(pc64[:, None, :, :].repeat(1, axis=1),
                              idx[..., None], axis=2) \
        if False else np.stack(
            [pc64[b][idx[b]] for b in range(B)])        # [B, P, 8, 3]
    d2c = ((cand - b64[None, :, None, :]) ** 2).sum(-1)  # [B, P, 8]
    # order by (d2, index) so ties pick the smaller n, like argmin
    ord_ = np.lexsort((idx, d2c), axis=-1)
    d2_sorted = np.take_along_axis(d2c, ord_, axis=-1)
    idx_sorted = np.take_along_axis(idx, ord_, axis=-1)
    best_idx = idx_sorted[..., 0]
    gap = d2_sorted[..., 1] - d2_sorted[..., 0]

    # 2) coverage-risk rows: device top-8 spread within the f32r noise band
    #    -> the true argmin may have been pushed out of the top-8. Full-row
    #    fp64 scan for those rows.
    spread = vals[..., 0].astype(np.float64) - vals[..., 7].astype(np.float64)
    cover_risk = spread < COVERAGE_EPS
    for b in range(B):
        rows = np.nonzero(cover_risk[b])[0]
        if rows.size == 0:
            continue
        d2_rows = ((b64[rows][:, None, :] - pc64[b][None, :, :]) ** 2).sum(-1)
        part = np.partition(d2_rows, 1, axis=1)
        best_idx[b, rows] = np.argmin(d2_rows, axis=1)
        gap[b, rows] = part[:, 1] - part[:, 0]

    # 3) knife-edge rows: fp64 top-2 gap so small that the reference's own
    #    fp32 rounding decides the winner. Recompute those rows with the
    #    reference's jnp ops (row-slicing the einsum is bitwise-stable).
    pc_j = jnp.asarray(point_cloud)
    pc_sq_j = jnp.sum(pc_j * pc_j, axis=-1)
    for b in range(B):
        rows = np.nonzero(gap[b] < KNIFE_EPS)[0]
        if rows.size == 0:
            continue
        bas_rows = jnp.asarray(basis[rows])
        b_sq_rows = jnp.sum(bas_rows * bas_rows, axis=-1)
        cross = jnp.einsum('bnd,pd->bpn', pc_j[b:b + 1], bas_rows)
        d2 = b_sq_rows[None, :, None] + pc_sq_j[b:b + 1][:, None, :] \
            - 2.0 * cross
        best_idx[b, rows] = np.asarray(jnp.argmin(d2, axis=-1))[0]
    return best_idx.astype(np.int64)


def _assemble(point_cloud, basis, best_idx):
    """Final gather + delta/dist with the reference's own jnp ops."""
    import jax.numpy as jnp
    pc_j = jnp.asarray(point_cloud)
    bas_j = jnp.asarray(basis)
    nearest = jnp.take_along_axis(pc_j, jnp.asarray(best_idx)[..., None],
                                  axis=1)
    deltas = nearest - bas_j[None, :, :]
    dists = jnp.sqrt(jnp.sum(deltas * deltas, axis=-1))
    out = jnp.concatenate([dists[..., None], deltas], axis=-1)
    return np.asarray(out).astype(np.float32)


def kernel(point_cloud, basis, _trace=False):
    point_cloud = np.asarray(point_cloud, dtype=np.float32)
    basis = np.asarray(basis, dtype=np.float32)
    assert point_cloud.shape == (B, N, D) and basis.shape == (P, D)
    vals, idx, res = _run_device(point_cloud, basis, trace=_trace)
    best_idx = _resolve_indices(point_cloud, basis, vals, idx)
    out = _assemble(point_cloud, basis, best_idx)
    if _trace:
        kernel.last_results = res
    return out
